# revision 1
# baseline (speedup 1.0000x reference)
"""Trainium2 Bass kernel for the RNN-T JointNetwork problem.

Computes log_softmax(tanh(cat(enc, pred)) @ W.T + b) over the vocab dim
for logits of shape [B=4, T=200, U=50, V=1024], fp32.

Strategy (data-parallel over the 800 flattened (b,t) rows, 100 per core):
  setup (per core, on device):
    teT  = tanh(encT_slice)          [512, 100]   (ACT)
    tpT  = tanh(predT_slice)         [512, 50]    (ACT)
    enc_p  = teT.T @ WeT             [100, 1024]  (PE, fp32)
    pred_b = tpT.T @ WpT + b         [50, 1024]   (PE, fp32)
  main loop over 40 row-tiles of 128 rows (row r = t*50 + u):
    x    = u_ind[k].T @ pred_b + t_ind[k].T @ enc_p   (PE -> PSUM, f32r;
           one-hot stationary operands do the broadcast-add, exactly)
    e,s  = exp(x), rowsum(e)         (ACT with accum_out)
    lse  = ln(s)                     (ACT)
    out  = x - lse                   (DVE tensor_scalar, PSUM -> SBUF)
    DMA out tile -> DRAM (round-robin over issue queues)
"""

import numpy as np

import concourse.bass as bass
import concourse.bacc as bacc
import concourse.tile as tile
from concourse import mybir
from concourse.bass_utils import run_bass_kernel_spmd

# Problem shapes (hardcoded per contract).
B, T, U, D, V = 4, 200, 50, 512, 1024
N_CORES = 8
BT = B * T                     # 800 flattened (b,t) rows
TPC = BT // N_CORES            # 100 (b,t) rows per core
ROWS = TPC * U                 # 5000 output rows per core
P = 128
NT = (ROWS + P - 1) // P       # 40 row-tiles per core
NV = V // 512                  # fp32 moving-operand free-dim limit is 512
DC = D // P                    # 4 contraction chunks of 128 for D=512

f32 = mybir.dt.float32
f32r = mybir.dt.float32r
bf16 = mybir.dt.bfloat16

# Which issue queues take the 40 output-tile DMAs, round-robin.
OUT_DMA_ENGINES = ("sync", "gpsimd")

TRACE = False
LAST_RESULT = None

_CACHE = {}


def _patch_act_tables():
    """Pin Exp/Ln to the one table set containing both, so the activation
    table-load pass never alternates sets inside the main loop.

    Claiming a set does NOT contain a function is always safe (it can only
    add loads); here it redirects Exp away from sets lacking Ln.
    """
    if getattr(bacc, "_joint_act_patch", False):
        return
    orig = bacc.get_activation_tables

    def patched(arch):
        t = dict(orig(arch))
        keep = "natural_log_exp_and_others"
        drop = {mybir.ActivationFunctionType.Exp, mybir.ActivationFunctionType.Ln}
        for name, fns in t.items():
            if name != keep:
                t[name] = set(fns) - drop
        return t

    bacc.get_activation_tables = patched
    bacc._joint_act_patch = True


def _build_indicators():
    """Per-row-tile one-hot stationary operands, shared by all cores.

    u_ind[u, k, c] = 1 iff row (128k+c) has u(row) == u  (row % 50)
    t_ind[t, k, c] = 1 iff row (128k+c) has t(row) == t  (row // 50)
    Columns for rows >= ROWS (tail of the last tile) are all-zero.
    """
    r = np.arange(NT * P)
    valid = r < ROWS
    u_ind = np.zeros((U, NT * P), dtype=np.float32)
    t_ind = np.zeros((TPC, NT * P), dtype=np.float32)
    u_ind[(r % U)[valid], r[valid]] = 1.0
    t_ind[(r // U)[valid], r[valid]] = 1.0
    return (
        np.ascontiguousarray(u_ind.reshape(U, NT, P)),
        np.ascontiguousarray(t_ind.reshape(TPC, NT, P)),
    )


def _build_program():
    _patch_act_tables()
    nc = bacc.Bacc("TRN2", target_bir_lowering=False, debug=False,
                   num_devices=N_CORES)

    encT = nc.dram_tensor("encT", [D, TPC], f32, kind="ExternalInput")
    predT = nc.dram_tensor("predT", [D, U], f32, kind="ExternalInput")
    # W in bf16: halves the 4MB load and runs setup matmuls at full PE rate;
    # the resulting ~1e-3 relative rounding of the logits is far inside the
    # output tolerance (outputs are O(1..10), fp32 pipeline elsewhere).
    wT = nc.dram_tensor("wT", [2 * D, V], bf16, kind="ExternalInput")
    bias = nc.dram_tensor("bias", [V], f32, kind="ExternalInput")
    u_ind = nc.dram_tensor("u_ind", [U, NT, P], bf16, kind="ExternalInput")
    t_ind = nc.dram_tensor("t_ind", [TPC, NT, P], bf16, kind="ExternalInput")
    out = nc.dram_tensor("out", [ROWS, V], f32, kind="ExternalOutput")

    with tile.TileContext(nc) as tc:
        with (
            tc.tile_pool(name="consts", bufs=1) as consts,
            tc.tile_pool(name="psum", bufs=4, space=bass.MemorySpace.PSUM) as psum,
            tc.tile_pool(name="scratch", bufs=2) as scratch,
            tc.tile_pool(name="outs", bufs=6) as outs,
            tc.tile_pool(name="small", bufs=8) as small,
        ):
            # ---- load constants / inputs (spread across DMA issue queues,
            #      wT chunked so setup matmuls can start before it finishes) ----
            wt_sb = consts.tile([P, 2 * DC, V], bf16)
            wT_r = wT.ap().rearrange("(c p) v -> p c v", p=P)
            for c in range(2 * DC):
                eng = nc.sync if c % 2 == 0 else nc.scalar
                eng.dma_start(out=wt_sb[:, c, :], in_=wT_r[:, c, :])
            encT_sb = consts.tile([P, DC, TPC], f32)
            nc.scalar.dma_start(out=encT_sb[:], in_=encT.ap().rearrange(
                "(c p) t -> p c t", p=P))
            predT_sb = consts.tile([P, DC, U], f32)
            nc.scalar.dma_start(out=predT_sb[:], in_=predT.ap().rearrange(
                "(c p) u -> p c u", p=P))
            # indicators split so the first tiles don't wait on the full 3MB
            KSPLIT = 6
            uind_sb = consts.tile([U, NT, P], bf16)
            nc.gpsimd.dma_start(out=uind_sb[:, :KSPLIT, :],
                                in_=u_ind.ap()[:, :KSPLIT, :])
            nc.gpsimd.dma_start(out=uind_sb[:, KSPLIT:, :],
                                in_=u_ind.ap()[:, KSPLIT:, :])
            tind_sb = consts.tile([TPC, NT, P], bf16)
            nc.gpsimd.dma_start(out=tind_sb[:, :KSPLIT, :],
                                in_=t_ind.ap()[:, :KSPLIT, :])
            nc.gpsimd.dma_start(out=tind_sb[:, KSPLIT:, :],
                                in_=t_ind.ap()[:, KSPLIT:, :])
            b_sb = consts.tile([1, V], f32)
            nc.scalar.dma_start(out=b_sb[:], in_=bias.ap().rearrange(
                "(p v) -> p v", p=1))
            ones_u = consts.tile([1, U], f32)
            nc.vector.memset(ones_u[:], 1.0)

            # ---- tanh of activations (transposed layout: d on partitions),
            #      bf16 out to pair with the bf16 weights in the setup GEMMs ----
            teT = consts.tile([P, DC, TPC], bf16)
            nc.scalar.activation(teT[:], encT_sb[:],
                                 mybir.ActivationFunctionType.Tanh)
            tpT = consts.tile([P, DC, U], bf16)
            nc.scalar.activation(tpT[:], predT_sb[:],
                                 mybir.ActivationFunctionType.Tanh)

            # ---- enc_p[t, v] = sum_d teT[d, t] * We[v, d] ----
            enc_p = consts.tile([TPC, V], bf16)
            enc_ps = psum.tile([TPC, V], f32, tag="x")
            for vc in range(NV):
                sl = slice(vc * 512, (vc + 1) * 512)
                for c in range(DC):
                    nc.tensor.matmul(enc_ps[:, sl], teT[:, c, :],
                                     wt_sb[:, c, sl],
                                     start=(c == 0), stop=(c == DC - 1))
            nc.vector.tensor_copy(enc_p[:], enc_ps[:])

            # ---- pred_b[u, v] = sum_d tpT[d, u] * Wp[v, d] + b[v] ----
            pred_b = consts.tile([U, V], bf16)
            pred_ps = psum.tile([U, V], f32, tag="x")
            for vc in range(NV):
                sl = slice(vc * 512, (vc + 1) * 512)
                for c in range(DC):
                    nc.tensor.matmul(pred_ps[:, sl], tpT[:, c, :],
                                     wt_sb[:, DC + c, sl],
                                     start=(c == 0), stop=False)
                nc.tensor.matmul(pred_ps[:, sl], ones_u[:], b_sb[:, sl],
                                 start=False, stop=True)
            nc.vector.tensor_copy(pred_b[:], pred_ps[:])

            # ---- main loop over row tiles ----
            dma_engines = [getattr(nc, e) for e in OUT_DMA_ENGINES]
            for k in range(NT):
                r0 = k * P
                rows = min(P, ROWS - r0)
                x_ps = psum.tile([P, V], f32, tag="x")
                # f32r: full-rate fp32 streaming on the PE.  The one-hot
                # stationary operand is exact in any precision; only the
                # pass-through of pred_b/enc_p values sees f32r rounding.
                for vc in range(NV):
                    sl = slice(vc * 512, (vc + 1) * 512)
                    nc.tensor.matmul(x_ps[:rows, sl],
                                     uind_sb[:, k, :rows],
                                     pred_b[:, sl],
                                     start=True, stop=False)
                for vc in range(NV):
                    sl = slice(vc * 512, (vc + 1) * 512)
                    nc.tensor.matmul(x_ps[:rows, sl],
                                     tind_sb[:, k, :rows],
                                     enc_p[:, sl],
                                     start=False, stop=True)
                sums = small.tile([P, 1], f32)
                escr = scratch.tile([P, V], f32)
                nc.scalar.activation(escr[:rows], x_ps[:rows],
                                     mybir.ActivationFunctionType.Exp,
                                     accum_out=sums[:rows])
                lse = small.tile([P, 1], f32)
                nc.scalar.activation(lse[:rows], sums[:rows],
                                     mybir.ActivationFunctionType.Ln)
                o = outs.tile([P, V], f32)
                nc.vector.tensor_scalar_sub(o[:rows], x_ps[:rows], lse[:rows])
                eng = dma_engines[k % len(dma_engines)]
                eng.dma_start(out=out.ap()[r0:r0 + rows, :], in_=o[:rows])

    nc.compile()
    return nc


def kernel(enc_out, pred_out, W, b):
    global LAST_RESULT
    enc_out = np.asarray(enc_out, dtype=np.float32)
    pred_out = np.asarray(pred_out, dtype=np.float32)
    W = np.asarray(W, dtype=np.float32)
    b = np.asarray(b, dtype=np.float32)

    if "nc" not in _CACHE:
        _CACHE["nc"] = _build_program()
        _CACHE["ind"] = _build_indicators()
    nc = _CACHE["nc"]
    u_ind, t_ind = _CACHE["ind"]

    import ml_dtypes
    wT = np.ascontiguousarray(W.T).astype(ml_dtypes.bfloat16)   # [2D, V]
    enc_flat = enc_out.reshape(BT, D)                 # [800, 512]

    in_maps = []
    for c in range(N_CORES):
        bt0 = c * TPC
        b_idx = bt0 // T
        in_maps.append({
            "encT": np.ascontiguousarray(enc_flat[bt0:bt0 + TPC].T),
            "predT": np.ascontiguousarray(pred_out[b_idx].T),
            "wT": wT,
            "bias": b,
            "u_ind": u_ind.astype(ml_dtypes.bfloat16),
            "t_ind": t_ind.astype(ml_dtypes.bfloat16),
        })

    res = run_bass_kernel_spmd(nc, in_maps, core_ids=list(range(N_CORES)),
                               trace=TRACE)
    LAST_RESULT = res
    full = np.concatenate([r["out"] for r in res.results], axis=0)
    return full.reshape(B, T, U, V)



# revision 7
# speedup vs baseline: 1.1819x; 1.1819x over previous
"""Trainium2 Bass kernel for the RNN-T JointNetwork problem.

Computes log_softmax(tanh(cat(enc, pred)) @ W.T + b) over the vocab dim
for logits of shape [B=4, T=200, U=50, V=1024], fp32.

Data-parallel over the 800 flattened (b,t) rows, 100 per core; 5000
output rows (r = t_local*50 + u) per core, 40 row-tiles of 128.

Two structural tricks vs. the naive per-tile pipeline:

1. Combined broadcast-add matmul.  x[r,:] = enc_p[t(r),:] + pred_b[u(r),:]
   is ONE one-hot matmul per 512-wide vocab chunk: the moving operand
   stacks pred_b (partitions 0-49) and a 78-row window of enc_p
   (partitions 50-127), and the stationary [128,128] one-hot selects
   u-row + t-row per output row.  Two windows (comb_A: t 0-77 for tiles
   0-29, comb_B: t 22-99 for tiles 30-39) cover every tile's t-span.
   Halves PE streaming vs. separate u/t matmuls.

2. Factorized log-sum-exp.  sum_v exp(e_v + p_v) = dot(exp(e), exp(p)),
   so lse needs NO per-tile exp: compute the projections transposed
   (v on partitions), exp once, contract with one tiny PE matmul into
   S[t,u], take ln, round-trip [100,50]->DRAM->[40,128], PE-transpose
   to [128,40].  Per tile the only non-PE work left is one fused
   (x - lse) PSUM->SBUF op, alternating DVE / ACT.
"""

import numpy as np

import concourse.bass as bass
import concourse.bacc as bacc
import concourse.tile as tile
from concourse import mybir
from concourse.bass_utils import run_bass_kernel_spmd

# Problem shapes (hardcoded per contract).
B, T, U, D, V = 4, 200, 50, 512, 1024
N_CORES = 8
BT = B * T                     # 800 flattened (b,t) rows
TPC = BT // N_CORES            # 100 (b,t) rows per core
ROWS = TPC * U                 # 5000 output rows per core
P = 128
NT = (ROWS + P - 1) // P       # 40 row-tiles per core
DC = D // P                    # 4 contraction chunks of 128 for D=512
NVC = V // P                   # 8 vocab chunks of 128
# Engine partition accesses must start 32-aligned, so the enc window
# lives at partitions 64..127 of the moving comb (pred_b at 0..49,
# 50..63 unused).  Window A = t 0..63 covers tiles 0..24 exactly
# (tile 24 ends at row 3199 = t 63); window B = t 64..99 covers the rest.
ENC_BASE = 64                  # comb partition where the enc window starts
ENC_WIN_B = 64                 # comb_B enc window starts at t=64
A_TILES = 25                   # tiles 0..24 use comb_A (t span <= 63)

f32 = mybir.dt.float32
bf16 = mybir.dt.bfloat16

OUT_DMA_ENGINES = ("sync", "scalar", "gpsimd")

TRACE = False
LAST_RESULT = None

_CACHE = {}


def _patch_act_tables():
    """Pin Exp/Ln/Identity to the one table set containing all three, so
    the activation table-load pass never alternates sets mid-kernel.
    Claiming a set does NOT contain a function is always safe."""
    if getattr(bacc, "_joint_act_patch", False):
        return
    orig = bacc.get_activation_tables

    def patched(arch):
        t = dict(orig(arch))
        keep = "natural_log_exp_and_others"
        drop = {
            mybir.ActivationFunctionType.Exp,
            mybir.ActivationFunctionType.Ln,
            mybir.ActivationFunctionType.Identity,
        }
        for name, fns in t.items():
            if name != keep:
                t[name] = set(fns) - drop
        return t

    bacc.get_activation_tables = patched
    bacc._joint_act_patch = True


def _build_hot():
    """Per-tile [128,128] one-hot stationaries (moving-comb row selectors).

    hot[p, k, m]: output row r = 128k + m takes moving-comb partition p
    with weight 1 when p is its u-row (p = u(r)) or its t-row
    (p = ENC_BASE + t(r) - win0(k)).  Columns for r >= ROWS are all-zero.
    """
    r = np.arange(NT * P)
    valid = r < ROWS
    u = r % U
    t = r // U
    win0 = np.where((r // P) < A_TILES, 0, ENC_WIN_B)
    hot = np.zeros((P, NT * P), dtype=np.float32)
    hot[u[valid], r[valid]] = 1.0
    hot[(ENC_BASE + t - win0)[valid], r[valid]] = 1.0
    return np.ascontiguousarray(hot.reshape(P, NT, P))


def _build_program():
    import ml_dtypes

    _patch_act_tables()
    nc = bacc.Bacc("TRN2", target_bir_lowering=False, debug=False,
                   num_devices=N_CORES)

    encT = nc.dram_tensor("encT", [D, TPC], f32, kind="ExternalInput")
    predT = nc.dram_tensor("predT", [D, U], f32, kind="ExternalInput")
    wT = nc.dram_tensor("wT", [2 * D, V], bf16, kind="ExternalInput")
    bias = nc.dram_tensor("bias", [V], f32, kind="ExternalInput")
    biasT = nc.dram_tensor("biasT", [P, NVC], f32, kind="ExternalInput")
    out = nc.dram_tensor("out", [ROWS, V], f32, kind="ExternalOutput")
    lse_dram = nc.dram_tensor("lse_scratch", [NT * P], f32, kind="Internal")

    hot_dram = nc.inline_tensor(
        _build_hot().astype(ml_dtypes.bfloat16), name="hot")
    eye_dram = nc.inline_tensor(
        np.eye(NT, dtype=np.float32), name="eye40")

    Act = mybir.ActivationFunctionType
    PSUM = bass.MemorySpace.PSUM

    with tile.TileContext(nc) as tc:
        with (
            tc.tile_pool(name="consts", bufs=1) as consts,
            tc.tile_pool(name="outs", bufs=6) as outs,
        ):
            # ---- input DMAs --------------------------------------------
            # W in 16 pieces ordered so the transposed GEMMs can start
            # as soon as the first d-column of pieces has landed.
            wt_sb = consts.tile([P, 2 * DC, V], bf16)
            wT_r = wT.ap().rearrange("(c p) v -> p c v", p=P)
            for half in range(2):
                sl = slice(half * 512, (half + 1) * 512)
                for c in range(2 * DC):
                    nc.sync.dma_start(out=wt_sb[:, c, sl], in_=wT_r[:, c, sl])

            tanh_in = consts.tile([P, DC, TPC + U], f32)
            nc.scalar.dma_start(
                out=tanh_in[:, :, 0:TPC],
                in_=encT.ap().rearrange("(c p) t -> p c t", p=P))
            nc.scalar.dma_start(
                out=tanh_in[:, :, TPC:TPC + U],
                in_=predT.ap().rearrange("(c p) u -> p c u", p=P))
            b_sb = consts.tile([1, V], f32)
            nc.scalar.dma_start(out=b_sb[:], in_=bias.ap().rearrange(
                "(p v) -> p v", p=1))
            bT_sb = consts.tile([P, NVC], f32)
            nc.scalar.dma_start(out=bT_sb[:], in_=biasT.ap())

            hot_sb = consts.tile([P, NT, P], bf16)
            for piece in range(4):
                ks = slice(piece * 10, (piece + 1) * 10)
                nc.gpsimd.dma_start(out=hot_sb[:, ks, :],
                                    in_=hot_dram.ap()[:, ks, :])
            eye_sb = consts.tile([NT, NT], f32)
            nc.gpsimd.dma_start(out=eye_sb[:], in_=eye_dram.ap())

            ones50 = consts.tile([1, U], f32)
            nc.vector.memset(ones50[:], 1.0)

            comb_A = consts.tile([P, V], bf16)
            comb_B = consts.tile([P, V], bf16)
            # zero the never-written partitions (50..63 of A, 100..127 of
            # B): the one-hot weight there is 0, but 0 x sbuf-garbage-NaN
            # would still poison the matmul.
            nc.vector.memset(comb_A[:], 0.0)
            nc.gpsimd.memset(comb_B[:], 0.0)

            # ---- tanh (one op, enc and pred stacked along free dim) ----
            tanh_bf = consts.tile([P, DC, TPC + U], bf16)
            nc.scalar.activation(tanh_bf[:], tanh_in[:], Act.Tanh)

            expT_sb = consts.tile([P, NVC, TPC + U], bf16)
            lse_all = consts.tile([TPC, U], f32)
            lse40 = consts.tile([NT, P], f32)
            neg_lse = consts.tile([P, NT], f32)

            # ---- transposed projections (v on partitions) + exp --------
            # projT[j][:, 0:100] = We_chunk^T tanh(enc)   (c = 0..3)
            # projT[j][:, 100:150] = Wp_chunk^T tanh(pred) (c = 4..7)
            # then expE = exp(.), expP = exp(. + b) straight out of PSUM.
            with tc.tile_pool(name="psA", bufs=2, space=PSUM) as psA:
                for j in range(NVC):
                    vsl = slice(j * P, (j + 1) * P)
                    projT = psA.tile([P, TPC + U], f32)
                    for c in range(DC):
                        nc.tensor.matmul(projT[:, 0:TPC],
                                         wt_sb[:, c, vsl],
                                         tanh_bf[:, c, 0:TPC],
                                         start=(c == 0), stop=(c == DC - 1))
                    for c in range(DC):
                        nc.tensor.matmul(projT[:, TPC:TPC + U],
                                         wt_sb[:, DC + c, vsl],
                                         tanh_bf[:, c, TPC:TPC + U],
                                         start=(c == 0), stop=(c == DC - 1))
                    nc.scalar.activation(expT_sb[:, j, 0:TPC],
                                         projT[:, 0:TPC], Act.Exp)
                    nc.scalar.activation(expT_sb[:, j, TPC:TPC + U],
                                         projT[:, TPC:TPC + U], Act.Exp,
                                         bias=bT_sb[:, j:j + 1])

            with tc.tile_pool(name="psB", bufs=1, space=PSUM) as psB:
                # ---- S[t,u] = sum_v expE[v,t] expP[v,u];  lse = ln S ----
                s_ps = psB.tile([TPC, U], f32)
                for j in range(NVC):
                    nc.tensor.matmul(s_ps[:], expT_sb[:, j, 0:TPC],
                                     expT_sb[:, j, TPC:TPC + U],
                                     start=(j == 0), stop=(j == NVC - 1))
                nc.scalar.activation(lse_all[:], s_ps[:], Act.Ln)

                # ---- reshape lse (t,u) -> row-tile layout [128, 40] ----
                # [100,50] -> flat r-order DRAM -> [40,128] -> PE transpose
                nc.sync.dma_start(
                    out=lse_dram.ap()[0:ROWS].rearrange("(t u) -> t u", t=TPC),
                    in_=lse_all[:])
                nc.sync.dma_start(
                    out=lse40[:],
                    in_=lse_dram.ap().rearrange("(k p) -> k p", k=NT))
                lse_t = psB.tile([P, NT], f32, name="lse_t")
                nc.tensor.transpose(lse_t[:], lse40[:], eye_sb[:])
                nc.vector.tensor_scalar_mul(neg_lse[:], lse_t[:], -1.0)

                # ---- normal-layout projections + bias ------------------
                with tc.tile_pool(name="psC", bufs=1, space=PSUM) as psC:
                    enc_ps = psC.tile([TPC, V], f32)
                    pred_ps = psC.tile([U, V], f32)
                    for half in range(2):
                        sl = slice(half * 512, (half + 1) * 512)
                        for c in range(DC):
                            nc.tensor.matmul(enc_ps[:, sl],
                                             tanh_bf[:, c, 0:TPC],
                                             wt_sb[:, c, sl],
                                             start=(c == 0), stop=(c == DC - 1))
                        for c in range(DC):
                            nc.tensor.matmul(pred_ps[:, sl],
                                             tanh_bf[:, c, TPC:TPC + U],
                                             wt_sb[:, DC + c, sl],
                                             start=(c == 0), stop=False)
                        nc.tensor.matmul(pred_ps[:, sl], ones50[:],
                                         b_sb[:, sl], start=False, stop=True)

                    # moving-comb assembly, DVE/ACT split (all partition
                    # starts 32-aligned: 0 or 64)
                    nc.vector.tensor_copy(comb_A[0:U, :], pred_ps[:])
                    nc.scalar.activation(comb_B[0:U, :], pred_ps[:],
                                         Act.Identity)
                    nc.vector.tensor_copy(comb_A[ENC_BASE:P, :],
                                          enc_ps[0:P - ENC_BASE, :])
                    nc.scalar.activation(comb_B[ENC_BASE:ENC_BASE + TPC - ENC_WIN_B, :],
                                         enc_ps[ENC_WIN_B:TPC, :],
                                         Act.Identity)

            # ---- main loop ---------------------------------------------
            dma_engines = [getattr(nc, e) for e in OUT_DMA_ENGINES]
            with tc.tile_pool(name="psX", bufs=4, space=PSUM) as psX:
                for k in range(NT):
                    r0 = k * P
                    rows = min(P, ROWS - r0)
                    comb = comb_A if k < A_TILES else comb_B
                    x_ps = psX.tile([P, V], f32, tag="x")
                    for half in range(2):
                        sl = slice(half * 512, (half + 1) * 512)
                        nc.tensor.matmul(x_ps[:, sl], hot_sb[:, k, :],
                                         comb[:, sl], start=True, stop=True)
                    o = outs.tile([P, V], f32)
                    if k % 2 == 0:
                        nc.vector.tensor_scalar_add(
                            o[:rows], x_ps[:rows], neg_lse[:rows, k:k + 1])
                    else:
                        nc.scalar.activation(
                            o[:rows], x_ps[:rows], Act.Identity,
                            bias=neg_lse[:rows, k:k + 1])
                    eng = dma_engines[k % len(dma_engines)]
                    eng.dma_start(out=out.ap()[r0:r0 + rows, :], in_=o[:rows])

    nc.compile()
    return nc


def kernel(enc_out, pred_out, W, b):
    global LAST_RESULT
    enc_out = np.asarray(enc_out, dtype=np.float32)
    pred_out = np.asarray(pred_out, dtype=np.float32)
    W = np.asarray(W, dtype=np.float32)
    b = np.asarray(b, dtype=np.float32)

    if "nc" not in _CACHE:
        _CACHE["nc"] = _build_program()
    nc = _CACHE["nc"]

    import ml_dtypes
    wT = np.ascontiguousarray(W.T).astype(ml_dtypes.bfloat16)   # [2D, V]
    bT = np.ascontiguousarray(b.reshape(NVC, P).T)              # [128, 8]
    enc_flat = enc_out.reshape(BT, D)                           # [800, 512]

    in_maps = []
    for c in range(N_CORES):
        bt0 = c * TPC
        b_idx = bt0 // T
        in_maps.append({
            "encT": np.ascontiguousarray(enc_flat[bt0:bt0 + TPC].T),
            "predT": np.ascontiguousarray(pred_out[b_idx].T),
            "wT": wT,
            "bias": b,
            "biasT": bT,
        })

    res = run_bass_kernel_spmd(nc, in_maps, core_ids=list(range(N_CORES)),
                               trace=TRACE)
    LAST_RESULT = res
    full = np.concatenate([r["out"] for r in res.results], axis=0)
    return full.reshape(B, T, U, V)


# revision 9
# speedup vs baseline: 1.2253x; 1.0367x over previous
"""Trainium2 Bass kernel for the RNN-T JointNetwork problem.

Computes log_softmax(tanh(cat(enc, pred)) @ W.T + b) over the vocab dim
for logits of shape [B=4, T=200, U=50, V=1024], fp32.

Data-parallel over the 800 flattened (b,t) rows, 100 per core; 5000
output rows (r = t_local*50 + u) per core, 40 row-tiles of 128.

Structure (all engines meet the ~57us/core HBM write floor):

1. One table set.  tanh is computed as 1 - 2/(exp(2x)+1) (ACT exp +
   DVE reciprocal), so the whole kernel uses only the
   natural_log_exp_and_others ACT set: a single ~2.7us table load that
   overlaps the input DMAs.

2. Transposed-first projections.  projT[v,(t|u)] = W_chunk^T tanh(...)
   runs as W streams in (v on partitions).  From PSUM it forks:
   exp -> expT (for lse), and a bf16 copy -> PE-transpose -> the
   normal-layout "comb" operand.  No second GEMM pass.

3. Combined broadcast-add matmul.  x[r,:] = pred_b[u(r),:] + b +
   enc_p[t(r),:] is ONE one-hot matmul per 512-wide vocab chunk:
   moving operand = comb (pred rows 0-49, bias row 50, enc window rows
   64-127), stationary = per-tile one-hot [128,128] with THREE ones per
   column (u-row, bias-row, t-row).  Windows t 0-63 (tiles 0-24) and
   t 64-99 (tiles 25-39) keep every engine access 32-partition-aligned.

4. Factorized log-sum-exp.  sum_v exp(e_v + p_v + b_v) =
   dot(exp(e), exp(p + b)): one tiny PE contraction -> S[t,u], ln,
   round-trip through DRAM to reshape [100,50] -> [40,128], PE
   transpose -> neg_lse [128,40].  Per tile the only non-PE work is one
   fused (x - lse) PSUM->SBUF op, alternating DVE / ACT, then the DMA.
"""

import numpy as np

import concourse.bass as bass
import concourse.bacc as bacc
import concourse.tile as tile
from concourse import mybir
from concourse.bass_utils import run_bass_kernel_spmd

# Problem shapes (hardcoded per contract).
B, T, U, D, V = 4, 200, 50, 512, 1024
N_CORES = 8
BT = B * T                     # 800 flattened (b,t) rows
TPC = BT // N_CORES            # 100 (b,t) rows per core
ROWS = TPC * U                 # 5000 output rows per core
P = 128
NT = (ROWS + P - 1) // P       # 40 row-tiles per core
DC = D // P                    # 4 contraction chunks of 128 for D=512
NVC = V // P                   # 8 vocab chunks of 128
TU = TPC + U                   # 150: t and u stacked on the free dim
BIAS_ROW = 50                  # comb partition holding the bias row
ENC_BASE = 64                  # comb partition where the enc window starts
ENC_WIN_B = 64                 # comb_B enc window starts at t=64
A_TILES = 25                   # tiles 0..24 use comb_A (t span <= 63)

f32 = mybir.dt.float32
bf16 = mybir.dt.bfloat16

OUT_DMA_ENGINES = ("sync", "scalar", "gpsimd")

TRACE = False
LAST_RESULT = None

_CACHE = {}


def _patch_act_tables():
    """Pin Exp/Ln/Identity to the one table set containing all three, so
    the activation table-load pass emits exactly one load.
    Claiming a set does NOT contain a function is always safe."""
    if getattr(bacc, "_joint_act_patch", False):
        return
    orig = bacc.get_activation_tables

    def patched(arch):
        t = dict(orig(arch))
        keep = "natural_log_exp_and_others"
        drop = {
            mybir.ActivationFunctionType.Exp,
            mybir.ActivationFunctionType.Ln,
            mybir.ActivationFunctionType.Identity,
        }
        for name, fns in t.items():
            if name != keep:
                t[name] = set(fns) - drop
        return t

    bacc.get_activation_tables = patched
    bacc._joint_act_patch = True


def _build_hot():
    """Per-tile [128,128] one-hot stationaries (moving-comb row selectors).

    hot[p, k, m]: output row r = 128k + m takes moving-comb partition p
    with weight 1 when p is its u-row (p = u(r)), the bias row
    (p = BIAS_ROW), or its t-row (p = ENC_BASE + t(r) - win0(k)).
    Columns for r >= ROWS are all-zero.
    """
    r = np.arange(NT * P)
    valid = r < ROWS
    u = r % U
    t = r // U
    win0 = np.where((r // P) < A_TILES, 0, ENC_WIN_B)
    hot = np.zeros((P, NT * P), dtype=np.float32)
    hot[u[valid], r[valid]] = 1.0
    hot[BIAS_ROW, valid] = 1.0
    hot[(ENC_BASE + t - win0)[valid], r[valid]] = 1.0
    return np.ascontiguousarray(hot.reshape(P, NT, P))


def _build_program():
    import ml_dtypes

    _patch_act_tables()
    nc = bacc.Bacc("TRN2", target_bir_lowering=False, debug=False,
                   num_devices=N_CORES)

    encT = nc.dram_tensor("encT", [D, TPC], f32, kind="ExternalInput")
    predT = nc.dram_tensor("predT", [D, U], f32, kind="ExternalInput")
    wT = nc.dram_tensor("wT", [2 * D, V], bf16, kind="ExternalInput")
    biasB = nc.dram_tensor("biasB", [1, V], bf16, kind="ExternalInput")
    biasT = nc.dram_tensor("biasT", [P, NVC], f32, kind="ExternalInput")
    out = nc.dram_tensor("out", [ROWS, V], f32, kind="ExternalOutput")
    lse_dram = nc.dram_tensor("lse_scratch", [NT * P], f32, kind="Internal")

    hot_dram = nc.inline_tensor(
        _build_hot().astype(ml_dtypes.bfloat16), name="hot")
    eye_bf_dram = nc.inline_tensor(
        np.eye(P, dtype=np.float32).astype(ml_dtypes.bfloat16), name="eyebf")
    eye40_dram = nc.inline_tensor(np.eye(NT, dtype=np.float32), name="eye40")

    Act = mybir.ActivationFunctionType
    PSUM = bass.MemorySpace.PSUM

    with tile.TileContext(nc) as tc:
        with (
            tc.tile_pool(name="consts", bufs=1) as consts,
            tc.tile_pool(name="outs", bufs=6) as outs,
        ):
            # ---- input DMAs --------------------------------------------
            # All DRAM APs use a (p c) row order (d = 4p + c within each
            # 512-row half) so every descriptor is one contiguous
            # per-partition run well above the 512B line-rate floor.
            tanh_in = consts.tile([P, DC, TU], f32)
            nc.sync.dma_start(
                out=tanh_in[:, :, 0:TPC],
                in_=encT.ap().rearrange("(p c) t -> p c t", p=P))
            nc.sync.dma_start(
                out=tanh_in[:, :, TPC:TU],
                in_=predT.ap().rearrange("(p c) u -> p c u", p=P))
            wt_sb = consts.tile([P, 2 * DC, V], bf16)
            nc.sync.dma_start(
                out=wt_sb[:, 0:DC, :],
                in_=wT.ap()[0:D, :].rearrange("(p c) v -> p c v", p=P))
            nc.sync.dma_start(
                out=wt_sb[:, DC:2 * DC, :],
                in_=wT.ap()[D:2 * D, :].rearrange("(p c) v -> p c v", p=P))

            hot_sb = consts.tile([P, NT, P], bf16)
            nc.gpsimd.dma_start(out=hot_sb[:, 0:NT // 2, :],
                                in_=hot_dram.ap()[:, 0:NT // 2, :])
            nc.gpsimd.dma_start(out=hot_sb[:, NT // 2:NT, :],
                                in_=hot_dram.ap()[:, NT // 2:NT, :])
            eye_bf = consts.tile([P, P], bf16)
            nc.gpsimd.dma_start(out=eye_bf[:], in_=eye_bf_dram.ap())
            eye40 = consts.tile([NT, NT], f32)
            nc.gpsimd.dma_start(out=eye40[:], in_=eye40_dram.ap())
            bT_sb = consts.tile([P, NVC], f32)
            nc.gpsimd.dma_start(out=bT_sb[:], in_=biasT.ap())

            comb_A = consts.tile([P, V], bf16)
            comb_B = consts.tile([P, V], bf16)
            # zero the never-written partitions (51..63; 100..127 of B):
            # their one-hot weight is 0, but 0 x sbuf-garbage-NaN would
            # still poison the matmul.
            nc.vector.memset(comb_A[:], 0.0)
            nc.gpsimd.memset(comb_B[:], 0.0)
            # bias row: DMA straight into partition 50 of each comb
            nc.gpsimd.dma_start(out=comb_A[BIAS_ROW:BIAS_ROW + 1, :],
                                in_=biasB.ap())
            nc.gpsimd.dma_start(out=comb_B[BIAS_ROW:BIAS_ROW + 1, :],
                                in_=biasB.ap())

            # ---- tanh via the exp table set ----------------------------
            # tanh(x) = 1 - 2/(exp(2x)+1): keeps the whole kernel on one
            # ACT table set (exp/ln/identity), saving two ~2.7us loads.
            y_exp = consts.tile([P, DC, TU], f32)
            nc.scalar.activation(y_exp[:], tanh_in[:], Act.Exp, scale=2.0)
            y_p1 = consts.tile([P, DC, TU], f32)
            nc.vector.tensor_scalar_add(y_p1[:], y_exp[:], 1.0)
            y_rc = consts.tile([P, DC, TU], f32)
            nc.vector.reciprocal(y_rc[:], y_p1[:])
            tanh_bf = consts.tile([P, DC, TU], bf16)
            nc.vector.tensor_scalar(tanh_bf[:], y_rc[:], -2.0, 1.0,
                                    mybir.AluOpType.mult,
                                    mybir.AluOpType.add)

            proj_sb = consts.tile([P, NVC, TU], bf16)
            expT_sb = consts.tile([P, NVC, TU], bf16)
            lse_all = consts.tile([TPC, U], f32)
            lse40 = consts.tile([NT, P], f32)
            neg_lse = consts.tile([P, NT], f32)

            # ---- per-vocab-chunk: projT GEMM -> exp / transpose --------
            with (
                tc.tile_pool(name="psA", bufs=2, space=PSUM) as psA,
                tc.tile_pool(name="psT", bufs=2, space=PSUM) as psT,
                tc.tile_pool(name="psB", bufs=1, space=PSUM) as psB,
            ):
                s_ps = psB.tile([TPC, U], f32)
                for j in range(NVC):
                    vsl = slice(j * P, (j + 1) * P)
                    projT = psA.tile([P, TU], f32)
                    for c in range(DC):
                        nc.tensor.matmul(projT[:, 0:TPC],
                                         wt_sb[:, c, vsl],
                                         tanh_bf[:, c, 0:TPC],
                                         start=(c == 0), stop=(c == DC - 1))
                    for c in range(DC):
                        nc.tensor.matmul(projT[:, TPC:TU],
                                         wt_sb[:, DC + c, vsl],
                                         tanh_bf[:, c, TPC:TU],
                                         start=(c == 0), stop=(c == DC - 1))
                    # lse branch: exp (pred side gets +b via the free
                    # affine), contract into S as soon as both halves exist
                    nc.scalar.activation(expT_sb[:, j, 0:TPC],
                                         projT[:, 0:TPC], Act.Exp)
                    nc.scalar.activation(expT_sb[:, j, TPC:TU],
                                         projT[:, TPC:TU], Act.Exp,
                                         bias=bT_sb[:, j:j + 1])
                    nc.tensor.matmul(s_ps[:], expT_sb[:, j, 0:TPC],
                                     expT_sb[:, j, TPC:TU],
                                     start=(j == 0), stop=(j == NVC - 1))
                    # comb branch: bf16 copy out of PSUM, PE-transpose to
                    # normal layout, slice into the comb windows
                    nc.vector.tensor_copy(proj_sb[:, j, :], projT[:])
                    tr_e = psT.tile([TPC, P], bf16, name="tr_e", tag="tr", bufs=3)
                    nc.tensor.transpose(tr_e[:], proj_sb[:, j, 0:TPC],
                                        eye_bf[:])
                    tr_p = psT.tile([U, P], bf16, name="tr_p", tag="tr", bufs=3)
                    nc.tensor.transpose(tr_p[:], proj_sb[:, j, TPC:TU],
                                        eye_bf[:])
                    nc.scalar.activation(comb_A[0:U, vsl], tr_p[:],
                                         Act.Identity)
                    nc.vector.tensor_copy(comb_A[ENC_BASE:P, vsl],
                                          tr_e[0:P - ENC_BASE, :])

                # ---- lse: ln(S), reshape to row-tile layout ------------
                nc.scalar.activation(lse_all[:], s_ps[:], Act.Ln)
                nc.scalar.dma_start(
                    out=lse_dram.ap()[0:ROWS].rearrange("(t u) -> t u", t=TPC),
                    in_=lse_all[:])
                nc.scalar.dma_start(
                    out=lse40[:],
                    in_=lse_dram.ap().rearrange("(k p) -> k p", k=NT))
                lse_t = psB.tile([P, NT], f32, name="lse_t")
                nc.tensor.transpose(lse_t[:], lse40[:], eye40[:])
                nc.vector.tensor_scalar_mul(neg_lse[:], lse_t[:], -1.0)

                # comb_B windows (only needed from tile 25 onward)
                for j in range(NVC):
                    vsl = slice(j * P, (j + 1) * P)
                    tr_e2 = psT.tile([TPC, P], bf16, name="tr_e2", tag="tr", bufs=3)
                    nc.tensor.transpose(tr_e2[:], proj_sb[:, j, 0:TPC],
                                        eye_bf[:])
                    tr_p2 = psT.tile([U, P], bf16, name="tr_p2", tag="tr", bufs=3)
                    nc.tensor.transpose(tr_p2[:], proj_sb[:, j, TPC:TU],
                                        eye_bf[:])
                    nc.scalar.activation(
                        comb_B[ENC_BASE:ENC_BASE + TPC - ENC_WIN_B, vsl],
                        tr_e2[ENC_WIN_B:TPC, :], Act.Identity)
                    nc.vector.tensor_copy(comb_B[0:U, vsl], tr_p2[:])

            # ---- main loop ---------------------------------------------
            dma_engines = [getattr(nc, e) for e in OUT_DMA_ENGINES]
            with tc.tile_pool(name="psX", bufs=4, space=PSUM) as psX:
                for k in range(NT):
                    r0 = k * P
                    rows = min(P, ROWS - r0)
                    comb = comb_A if k < A_TILES else comb_B
                    x_ps = psX.tile([P, V], f32, tag="x")
                    for half in range(2):
                        sl = slice(half * 512, (half + 1) * 512)
                        nc.tensor.matmul(x_ps[:, sl], hot_sb[:, k, :],
                                         comb[:, sl], start=True, stop=True)
                    o = outs.tile([P, V], f32)
                    if k % 2 == 0:
                        nc.vector.tensor_scalar_add(
                            o[:rows], x_ps[:rows], neg_lse[:rows, k:k + 1])
                    else:
                        nc.scalar.activation(
                            o[:rows], x_ps[:rows], Act.Identity,
                            bias=neg_lse[:rows, k:k + 1])
                    eng = dma_engines[k % len(dma_engines)]
                    eng.dma_start(out=out.ap()[r0:r0 + rows, :], in_=o[:rows])

    nc.compile()
    return nc


def kernel(enc_out, pred_out, W, b):
    global LAST_RESULT
    enc_out = np.asarray(enc_out, dtype=np.float32)
    pred_out = np.asarray(pred_out, dtype=np.float32)
    W = np.asarray(W, dtype=np.float32)
    b = np.asarray(b, dtype=np.float32)

    if "nc" not in _CACHE:
        _CACHE["nc"] = _build_program()
    nc = _CACHE["nc"]

    import ml_dtypes
    wT = np.ascontiguousarray(W.T).astype(ml_dtypes.bfloat16)   # [2D, V]
    bB = np.ascontiguousarray(b.reshape(1, V)).astype(ml_dtypes.bfloat16)
    bT = np.ascontiguousarray(b.reshape(NVC, P).T)              # [128, 8]
    enc_flat = enc_out.reshape(BT, D)                           # [800, 512]

    in_maps = []
    for c in range(N_CORES):
        bt0 = c * TPC
        b_idx = bt0 // T
        in_maps.append({
            "encT": np.ascontiguousarray(enc_flat[bt0:bt0 + TPC].T),
            "predT": np.ascontiguousarray(pred_out[b_idx].T),
            "wT": wT,
            "biasB": bB,
            "biasT": bT,
        })

    res = run_bass_kernel_spmd(nc, in_maps, core_ids=list(range(N_CORES)),
                               trace=TRACE)
    LAST_RESULT = res
    full = np.concatenate([r["out"] for r in res.results], axis=0)
    return full.reshape(B, T, U, V)
